# revision 20
# baseline (speedup 1.0000x reference)
"""Re-attention transformer block on 8 trn2 NeuronCores (batch-parallel).

Math (per batch element, h=8 heads, d=64, n=1024, dim=512):
  qkv = x @ w_qkv ; per-head dots = q k^T/8 ; p = softmax_j(dots)
  attn2 = einsum('hij,hg->gij', p, W) ; LN over g ; out = reattn @ v ; proj.

Device layout: JI orientation (j on partitions, i on free) everywhere.
Host folds: scale 1/8 into w_q; mix-centering (W - rowmean) into the
scaled-identity mix weights; ln_g into the A.V psum->sbuf copy; ln_b via a
rank-1 K=1 matmul into the A.V accumulation.
"""
import numpy as np
import ml_dtypes
from contextlib import ExitStack

import concourse.bass as bass
import concourse.bacc as bacc
import concourse.mybir as mybir
import concourse.tile as tile
from concourse.bass_utils import run_bass_kernel_spmd

BF = mybir.dt.bfloat16
F32 = mybir.dt.float32
AF = mybir.ActivationFunctionType

H, D, N, DIM = 8, 64, 1024, 512
INNER = H * D
LN_EPS = 1e-5

_cached = None


def _build():
    nc = bacc.Bacc()
    xt = nc.declare_dram_parameter("xt", [DIM, N], BF, isOutput=False)
    wqk = nc.declare_dram_parameter("wqk", [DIM, 2 * INNER], BF, isOutput=False)
    wv = nc.declare_dram_parameter("wv", [DIM, INNER], BF, isOutput=False)
    wout = nc.declare_dram_parameter("wout", [INNER, DIM], BF, isOutput=False)
    wid = nc.declare_dram_parameter("wid", [H * H, 128, 128], BF, isOutput=False)
    lngcol = nc.declare_dram_parameter("lngcol", [4, 128], F32, isOutput=False)
    lnbpat = nc.declare_dram_parameter("lnbpat", [1, INNER], F32, isOutput=False)
    boutp = nc.declare_dram_parameter("boutp", [1, DIM], F32, isOutput=False)
    out = nc.declare_dram_parameter("out", [N, DIM], F32, isOutput=True)

    with tile.TileContext(nc) as tc:
        with ExitStack() as ctx, \
                nc.allow_low_precision(reason="bf16 attention internals"):
            kern(ctx, tc, xt, wqk, wv, wout, wid, lngcol, lnbpat, boutp, out)
    nc.compile()
    return nc


def _fix_mm_waits(nc):
    """Several engine ISA structs carry a single sync-wait slot; Tile
    sometimes emits two. Hoist extra waits onto the nearest preceding
    same-engine instruction with a free slot (in-order => conservative)."""
    def _num(name):
        try:
            return int(name.rsplit("-", 1)[1])
        except (IndexError, ValueError):
            return -1
    items = sorted(nc.inst_map.items(), key=lambda kv: _num(kv[0]))
    per_engine = {}
    for _, i in items:
        eng = getattr(i, "engine", None)
        if eng is None:
            continue
        per_engine.setdefault(eng, []).append(i)
    fixed = 0
    for eng, order in per_engine.items():
        for k, i in enumerate(order):
            si = i.sync_info
            if not si or not si.on_wait or len(si.on_wait) <= 1:
                continue
            waits = list(si.on_wait)
            keep = waits[-1:]
            moved = waits[:-1]
            j = k - 1
            while moved and j >= 0:
                prev = order[j]
                psi = prev.sync_info
                if psi is None:
                    w = moved.pop()
                    prev.sync_info = mybir.SyncInfo(on_wait=[w], on_update=[])
                elif not psi.on_wait:
                    psi.on_wait.append(moved.pop())
                j -= 1
            assert not moved, f"could not hoist waits for {i.name}"
            si.on_wait.clear()
            si.on_wait.extend(keep)
            fixed += 1


def kern(ctx, tc, xt, wqk, wv, wout, wid, lngcol, lnbpat, boutp, out):
    nc = tc.nc
    singles = ctx.enter_context(tc.tile_pool(name="singles", bufs=1))
    big = ctx.enter_context(tc.tile_pool(name="big", bufs=1))

    # ---- load small persistent constants into SBUF ----
    wout_sb = singles.tile([128, 4, DIM], BF, tag="wout")
    nc.sync.dma_start(out=wout_sb, in_=wout.rearrange("(a p) n -> p a n", p=128))
    lng_sb = singles.tile([128, 4], F32, tag="lng")
    nc.sync.dma_start(out=lng_sb, in_=lngcol.rearrange("a p -> p a"))
    lnb_sb = singles.tile([1, INNER], F32, tag="lnb")
    nc.sync.dma_start(out=lnb_sb, in_=lnbpat[:, :])
    bout_b = singles.tile([128, DIM], F32, tag="boutb")
    bout_bcast = bass.AP(tensor=boutp, offset=0, ap=[[0, 128], [1, DIM]])
    nc.gpsimd.dma_start(out=bout_b, in_=bout_bcast)
    ones_sb = singles.tile([128, 1], BF, tag="ones")
    nc.vector.memset(ones_sb, 1.0)
    eps_sb = singles.tile([128, 1], F32, tag="eps")
    nc.vector.memset(eps_sb, LN_EPS)

    # ---- qkv projections (xt/wqk/wv scoped: freed after this phase) ----
    v_sb = singles.tile([128, 8, INNER], BF, tag="v")  # [n%128, n//128, (g d)]
    qkp_cm = tc.tile_pool(name="qkp", bufs=1)
    qkp = qkp_cm.__enter__()
    qk_sb = qkp.tile([128, 8, N], BF, tag="qk")  # [f%128, f//128, n]
    with tc.tile_pool(name="ldp", bufs=1) as ldp, \
         tc.tile_pool(name="pp", bufs=3, space="PSUM") as pp:
        xt_sb = ldp.tile([128, 4, N], BF, tag="xt")
        nc.sync.dma_start(out=xt_sb, in_=xt.rearrange("(a p) n -> p a n", p=128))
        wqk_sb = ldp.tile([128, 4, 2 * INNER], BF, tag="wqk")
        nc.sync.dma_start(out=wqk_sb, in_=wqk.rearrange("(a p) n -> p a n", p=128))
        wv_sb = ldp.tile([128, 4, INNER], BF, tag="wv")
        nc.sync.dma_start(out=wv_sb, in_=wv.rearrange("(a p) n -> p a n", p=128))
        for mt in range(8):  # qk^T: out rows f
            for ic in range(2):
                ps = pp.tile([128, 512], F32, tag="ps")
                for kt in range(4):
                    nc.tensor.matmul(
                        ps,
                        wqk_sb[:, kt, mt * 128:(mt + 1) * 128],
                        xt_sb[:, kt, ic * 512:(ic + 1) * 512],
                        start=(kt == 0), stop=(kt == 3))
                nc.scalar.copy(qk_sb[:, mt, ic * 512:(ic + 1) * 512], ps)
        for mt in range(8):  # v: out rows n
            ps = pp.tile([128, INNER], F32, tag="ps2")
            for kt in range(4):
                nc.tensor.matmul(
                    ps, xt_sb[:, kt, mt * 128:(mt + 1) * 128], wv_sb[:, kt, :],
                    start=(kt == 0), stop=(kt == 3))
            nc.scalar.copy(v_sb[:, mt, :], ps)

    def qT(h):  # [64, N] rows of q for head h
        return qk_sb[(h % 2) * 64:(h % 2) * 64 + 64, h // 2, :]

    def kT(h):
        hh = h + 8
        return qk_sb[(hh % 2) * 64:(hh % 2) * 64 + 64, hh // 2, :]

    # ---- dots + exp (JI: [j,i]), per (head, j-tile) ----
    e_sb = big.tile([128, H, 8, N], BF, tag="e")  # e/p/reattn storage
    with tc.tile_pool(name="dp", bufs=6, space="PSUM") as dp:
        for h in range(H):
            for jt in range(8):
                for ic in range(2):
                    ps = dp.tile([128, 512], F32, tag="dots")
                    nc.tensor.matmul(
                        ps,
                        kT(h)[:, jt * 128:(jt + 1) * 128],
                        qT(h)[:, ic * 512:(ic + 1) * 512],
                        start=True, stop=True)
                    nc.scalar.activation(
                        e_sb[:, h, jt, ic * 512:(ic + 1) * 512], ps, AF.Exp)
    qkp_cm.__exit__(None, None, None)

    # ---- softmax denominators: s[h,i] = sum_j e ; rs = 1/s ----
    rs_dram = nc.dram_tensor("rs_scratch", [H, N], BF)
    rsp_cm = tc.tile_pool(name="rsp", bufs=1)
    rsp = rsp_cm.__enter__()
    rs_sb = rsp.tile([128, 8, N], BF, tag="rsb")  # broadcast recips
    with tc.tile_pool(name="sp", bufs=2, space="PSUM") as sp, \
         tc.tile_pool(name="st", bufs=1) as st:
        for h in range(H):
            srow = st.tile([1, N], F32, tag="srow")
            for ic in range(2):
                ps = sp.tile([1, 512], F32, tag="s")
                for jt in range(8):
                    nc.tensor.matmul(
                        ps, ones_sb, e_sb[:, h, jt, ic * 512:(ic + 1) * 512],
                        start=(jt == 0), stop=(jt == 7))
                nc.scalar.copy(srow[:, ic * 512:(ic + 1) * 512], ps)
            rrow = st.tile([1, N], BF, tag="rrow")
            nc.vector.reciprocal(rrow, srow)
            nc.sync.dma_start(out=rs_dram[h:h + 1, :], in_=rrow)
            rbc = bass.AP(tensor=rs_dram, offset=h * N,
                          ap=[[0, 128], [1, N]])
            nc.gpsimd.dma_start(out=rs_sb[:, h, :], in_=rbc)

    # ---- normalize: p = e * rs (rows broadcast over j) ----
    for h in range(H):
        for jt in range(8):
            nc.vector.tensor_mul(e_sb[:, h, jt, :], e_sb[:, h, jt, :],
                                 rs_sb[:, h, :])
    rsp_cm.__exit__(None, None, None)

    # ---- mix + LN, per (jt, ic) block; write reattn back over e ----
    with tc.tile_pool(name="mp", bufs=1, space="PSUM") as mp, \
         tc.tile_pool(name="mt", bufs=1) as mtp:
        wid_sb = mtp.tile([128, H * H, 128], BF, tag="wid")
        nc.sync.dma_start(out=wid_sb, in_=wid.rearrange("w p n -> p w n"))
        for jt in range(8):
            for ic in range(2):
                psg = []
                for g in range(H):
                    ps = mp.tile([128, 512], F32, tag=f"mix{g}")
                    for h in range(H):
                        nc.tensor.matmul(
                            ps, wid_sb[:, g * H + h, :],
                            e_sb[:, h, jt, ic * 512:(ic + 1) * 512],
                            start=(h == 0), stop=(h == 7))
                    psg.append(ps)
                a2c = mtp.tile([128, H, 512], BF, tag="a2c")
                for g in range(H):
                    nc.scalar.copy(a2c[:, g, :], psg[g])
                # var = mean_g a2c^2 (tree-sum of squares on DVE)
                sq = mtp.tile([128, H, 512], BF, tag="sq")
                for g in range(H):
                    nc.vector.tensor_mul(sq[:, g, :], a2c[:, g, :], a2c[:, g, :])
                for g in range(4):
                    nc.vector.tensor_add(sq[:, g, :], sq[:, g, :], sq[:, g + 4, :])
                for g in range(2):
                    nc.vector.tensor_add(sq[:, g, :], sq[:, g, :], sq[:, g + 2, :])
                nc.vector.tensor_add(sq[:, 0, :], sq[:, 0, :], sq[:, 1, :])
                sd = mtp.tile([128, 512], F32, tag="sd")
                nc.scalar.activation(sd, sq[:, 0, :], AF.Sqrt,
                                     bias=eps_sb, scale=0.125)
                rv = mtp.tile([128, 512], BF, tag="rv")
                nc.vector.reciprocal(rv, sd)
                for g in range(H):
                    nc.vector.tensor_mul(
                        e_sb[:, g, jt, ic * 512:(ic + 1) * 512],
                        a2c[:, g, :], rv)

    # ---- ln_b rank-1 prep: vsum[f] = sum_n v ; lnbv = ln_b[g]*vsum ----
    lnbv = singles.tile([1, INNER], BF, tag="lnbv")
    with tc.tile_pool(name="vp", bufs=2, space="PSUM") as vp, \
         tc.tile_pool(name="vt", bufs=2) as vt:
        ps = vp.tile([1, INNER], F32, tag="vs")
        for jt in range(8):
            nc.tensor.matmul(ps, ones_sb, v_sb[:, jt, :],
                             start=(jt == 0), stop=(jt == 7))
        vs = vt.tile([1, INNER], F32, tag="vsum")
        nc.scalar.copy(vs, ps)
        nc.vector.tensor_mul(lnbv, vs, lnb_sb)
    ones_i = singles.tile([1, N], BF, tag="onesi")
    nc.vector.memset(ones_i, 1.0)

    # ---- A.V: outT[(g,d), i] = sum_j reattn[j,i] v[j,(g,d)] + ln_b term ----
    otp = ctx.enter_context(tc.tile_pool(name="otp", bufs=1))
    ot_sb = otp.tile([128, 4, N], BF, tag="ot")
    with tc.tile_pool(name="ap", bufs=4, space="PSUM") as ap:
        for t in range(4):  # 128 rows = heads 2t,2t+1
            for ic in range(2):
                isl = slice(ic * 512, (ic + 1) * 512)
                ps = ap.tile([128, 512], F32, tag="av")
                for gg in range(2):
                    g = 2 * t + gg
                    sl = slice(gg * 64, gg * 64 + 64)
                    nc.tensor.matmul(ps[sl, :],
                                     lnbv[:, g * 64:(g + 1) * 64],
                                     ones_i[:, isl], start=True, stop=False)
                    for jt in range(8):
                        nc.tensor.matmul(
                            ps[sl, :],
                            v_sb[:, jt, g * 64:(g + 1) * 64],
                            e_sb[:, g, jt, isl],
                            start=False, stop=(jt == 7))
                nc.vector.tensor_scalar_mul(ot_sb[:, t, isl], ps,
                                            lng_sb[:, t:t + 1])

    # ---- projection: out[n, dim] = outT^T @ w_out + b_out ----
    with tc.tile_pool(name="fp", bufs=4, space="PSUM") as fp, \
         tc.tile_pool(name="ft", bufs=4) as ft:
        for ntile in range(8):
            ps = fp.tile([128, DIM], F32, tag="fin")
            for kt in range(4):
                nc.tensor.matmul(
                    ps, ot_sb[:, kt, ntile * 128:(ntile + 1) * 128],
                    wout_sb[:, kt, :], start=(kt == 0), stop=(kt == 3))
            fo = ft.tile([128, DIM], F32, tag="fo")
            nc.vector.tensor_add(fo, ps, bout_b)
            nc.sync.dma_start(out=out[ntile * 128:(ntile + 1) * 128, :], in_=fo)


def kernel(x, w_qkv, reattn_w, ln_g, ln_b, w_out, b_out):
    global _cached
    x = np.asarray(x, np.float32)
    w_qkv = np.asarray(w_qkv, np.float32)
    reattn_w = np.asarray(reattn_w, np.float32)
    ln_g = np.asarray(ln_g, np.float32)
    ln_b = np.asarray(ln_b, np.float32)
    w_out = np.asarray(w_out, np.float32)
    b_out = np.asarray(b_out, np.float32)
    b = x.shape[0]
    bf = ml_dtypes.bfloat16

    wq = (w_qkv[:, :INNER] * (D ** -0.5)).astype(bf)  # fold scale into q
    wk = w_qkv[:, INNER:2 * INNER].astype(bf)
    wqk = np.concatenate([wq, wk], axis=1)
    wv = w_qkv[:, 2 * INNER:].astype(bf)
    # centered mix weights -> scaled identities [g*8+h, 128, 128]
    wt = reattn_w - reattn_w.mean(axis=1, keepdims=True)  # [h,g]
    eye = np.eye(128, dtype=np.float32)
    wid = np.stack([eye * wt[h, g] for g in range(H) for h in range(H)])
    wid = wid.astype(bf)
    lngcol = np.repeat(ln_g, D).reshape(4, 128).astype(np.float32)
    lnbpat = np.repeat(ln_b, D).reshape(1, INNER).astype(np.float32)

    if _cached is None:
        _cached = _build()
    nc = _cached

    shared = dict(
        wqk=np.ascontiguousarray(wqk),
        wv=np.ascontiguousarray(wv),
        wout=np.ascontiguousarray(w_out.astype(bf)),
        wid=np.ascontiguousarray(wid),
        lngcol=lngcol, lnbpat=lnbpat,
        boutp=np.ascontiguousarray(b_out.reshape(1, DIM)),
    )
    in_maps = []
    for i in range(8):
        m = dict(shared)
        m["xt"] = np.ascontiguousarray(x[i % b].T.astype(bf))
        in_maps.append(m)

    res = run_bass_kernel_spmd(nc, in_maps, list(range(8))).results
    return np.stack([np.asarray(res[i]["out"], np.float32) for i in range(b)])
